# revision 3
# baseline (speedup 1.0000x reference)
"""FAMoE layer Trainium2 kernel.

Math (per batch row b of x [B, H, L]):
  rfft over L is a matmul with fixed DFT bases R/I [L, F];
  gating = softmax(MLP(mean_h |X|)) @ band-mask -> per-frequency weight w [F];
  irfft(X * w) is a matmul with bases A/B2 [L, F] whose rows we pre-scale by w.
Output is [B, L, H] (transposed), which falls out naturally because the whole
pipeline runs in a frequency/L-on-partitions orientation:

  per b:  x tiles [128 rows, 2*50]  --PE transpose (matmul vs identity)-->
          xT [100, 512] (two 50-row h-blocks stacked)  --D matmul (RI2)-->
          C/S [116, 512] in PSUM (C at partitions 0-51, S at 64-115)
          --evac/square/add/sqrt+accum--> gating input [52, 1] per b
          ...chunk MLP...  --per-b scaled irfft basis (ABB*w) matmul-->
          out [114, 512] (l of h-block A at partitions 0-49, B at 64-113).

All partition-block bases are 0 or 64 (hardware requires 32-aligned bases).
Sharding: pure data parallel, batch dim split across 8 cores.
"""

import sys

sys.path.insert(0, "/opt/trn_rl_repo")

import numpy as np

N_CORES = 8
B, H, L = 2048, 1024, 50
F = 26
E = 8
BS = B // N_CORES          # 256 batch rows per core
NB = 16                    # MLP chunk size (batch rows per gating batch)
QUAD = 4                   # b rows per input DMA

_cache = {}


def _dft_consts():
    l = np.arange(L)[:, None].astype(np.float64)
    f = np.arange(F)[None, :].astype(np.float64)
    ang = 2.0 * np.pi * l * f / L
    R = np.cos(ang)                      # [L, F] rfft real basis
    I = -np.sin(ang)                     # [L, F] rfft imag basis
    c = np.full(F, 2.0)
    c[0] = 1.0
    c[F - 1] = 1.0
    A = c[None, :] * np.cos(ang) / L     # [L, F] irfft cos basis
    B2 = -c[None, :] * np.sin(ang) / L   # [L, F] irfft sin basis
    return R, I, A, B2


def _build_ri2(R, I):
    # lhsT for the forward DFT: [100, 116]
    # rows 0-49: l of h-block A; rows 50-99: l of h-block B
    # cols 0-25 C_A | 26-51 C_B | 52-63 zero | 64-89 S_A | 90-115 S_B
    RI2 = np.zeros((100, 116), np.float64)
    RI2[0:50, 0:26] = R
    RI2[50:100, 26:52] = R
    RI2[0:50, 64:90] = I
    RI2[50:100, 90:116] = I
    return RI2


def _build_abb(A, B2):
    # base lhsT for the inverse DFT: [116, 114]
    # rows (k) match the C/S PSUM partition layout; cols 0-49 out-l of block
    # A, 50-63 zero, 64-113 out-l of block B. Per-b this is row-scaled by w.
    ABB = np.zeros((116, 114), np.float64)
    ABB[0:26, 0:50] = A.T
    ABB[26:52, 64:114] = A.T
    ABB[64:90, 0:50] = B2.T
    ABB[90:116, 64:114] = B2.T
    return ABB


def _build_sel():
    # [26, 116] selector: wrep[q] = w[f(q)] for the four 26-blocks at 0/26/64/90
    S = np.zeros((26, 116), np.float32)
    for base in (0, 26, 64, 90):
        S[np.arange(26), base + np.arange(26)] = 1.0
    return S


def _build_program(bs, nb):
    from concourse import bacc, bass, mybir, tile

    f32 = mybir.dt.float32
    bf16 = mybir.dt.bfloat16

    nc = bacc.Bacc("TRN2", target_bir_lowering=False, debug=False)

    x_d = nc.dram_tensor("x", [bs, H, L], f32, kind="ExternalInput")
    out_d = nc.dram_tensor("out", [bs, L, H], f32, kind="ExternalOutput")
    ident_d = nc.dram_tensor("ident", [128, 128], bf16, kind="ExternalInput")
    ri2_d = nc.dram_tensor("ri2", [100, 116], bf16, kind="ExternalInput")
    abb_d = nc.dram_tensor("abb", [116, 114], bf16, kind="ExternalInput")
    w1f_d = nc.dram_tensor("w1f", [2 * F, F], f32, kind="ExternalInput")
    b1_d = nc.dram_tensor("b1c", [F, 1], f32, kind="ExternalInput")
    w2_d = nc.dram_tensor("w2", [F, E], f32, kind="ExternalInput")
    b2_d = nc.dram_tensor("b2c", [E, 1], f32, kind="ExternalInput")
    mask_d = nc.dram_tensor("mask", [E, F], f32, kind="ExternalInput")
    ones8_d = nc.dram_tensor("ones8", [E, 1], f32, kind="ExternalInput")
    ones8r_d = nc.dram_tensor("ones8r", [1, E], f32, kind="ExternalInput")
    sel_d = nc.dram_tensor("sel", [F, 116], f32, kind="ExternalInput")

    n_chunk = bs // nb
    assert bs % nb == 0 and nb % QUAD == 0

    with tile.TileContext(nc) as tc:
        with (
            tc.tile_pool(name="consts", bufs=1) as cpool,
            tc.tile_pool(name="xin", bufs=3) as xpool,
            tc.tile_pool(name="xt", bufs=3) as xtpool,
            tc.tile_pool(name="cs", bufs=nb + 4) as cspool,
            tc.tile_pool(name="mag", bufs=3) as magpool,
            tc.tile_pool(name="outs", bufs=3) as opool,
            tc.tile_pool(name="gat", bufs=2) as gpool,
            tc.tile_pool(name="lo", bufs=3) as lopool,
            tc.tile_pool(name="ps_xt", bufs=2, space="PSUM") as ps_xt,
            tc.tile_pool(name="ps_cs", bufs=2, space="PSUM") as ps_cs,
            tc.tile_pool(name="ps_out", bufs=2, space="PSUM") as ps_out,
            tc.tile_pool(name="ps_sm", bufs=2, space="PSUM") as ps_sm,
        ):
            ident = cpool.tile([128, 128], bf16)
            ri2 = cpool.tile([100, 116], bf16)
            abb = cpool.tile([116, 114], bf16)
            w1f = cpool.tile([2 * F, F], f32)
            b1 = cpool.tile([F, 1], f32)
            w2 = cpool.tile([F, E], f32)
            b2 = cpool.tile([E, 1], f32)
            mask = cpool.tile([E, F], f32)
            ones8 = cpool.tile([E, 1], f32)
            ones8r = cpool.tile([1, E], f32)
            sel = cpool.tile([F, 116], f32)
            for t, d in [
                (ident, ident_d), (ri2, ri2_d), (abb, abb_d), (w1f, w1f_d),
                (b1, b1_d), (w2, w2_d), (b2, b2_d), (mask, mask_d),
                (ones8, ones8_d), (ones8r, ones8r_d), (sel, sel_d),
            ]:
                nc.sync.dma_start(t[:], d[:])

            Sq = mybir.ActivationFunctionType.Square
            Sqrt = mybir.ActivationFunctionType.Sqrt
            Copy = mybir.ActivationFunctionType.Copy
            Relu = mybir.ActivationFunctionType.Relu
            Exp = mybir.ActivationFunctionType.Exp
            MUL = mybir.AluOpType.mult
            ADD = mybir.AluOpType.add

            for c in range(n_chunk):
                gbuf = gpool.tile([52, nb], f32, tag="gbuf")
                cs_tiles = []
                for j in range(nb):
                    bb = c * nb + j
                    # ---- input DMA: 4 b-rows per SWDGE cast DMA ----
                    if j % QUAD == 0:
                        x_nat = xpool.tile([128, QUAD * 400], bf16, tag="xnat")
                        src = x_d[bb : bb + QUAD].rearrange(
                            "b (t pair p) l -> p b t pair l", t=4, pair=2, p=128
                        )
                        dst = x_nat[:, :].rearrange(
                            "p (b t pair l) -> p b t pair l", b=QUAD, t=4, pair=2, l=50
                        )
                        nc.gpsimd.dma_start(out=dst, in_=src)
                    qoff = (j % QUAD) * 400

                    # ---- transpose via PE (x tile stationary, identity moving) ----
                    p_xt = ps_xt.tile([100, 512], f32, tag="pxt")
                    for t in range(4):
                        nc.tensor.matmul(
                            p_xt[:, 128 * t : 128 * t + 128],
                            x_nat[:, qoff + 100 * t : qoff + 100 * t + 100],
                            ident[:],
                        )
                    xt = xtpool.tile([100, 512], bf16, tag="xt")
                    nc.scalar.activation(xt[:], p_xt[:], Copy)

                    # ---- forward DFT ----
                    p_cs = ps_cs.tile([116, 512], f32, tag="pcs")
                    nc.tensor.matmul(p_cs[:], ri2[:], xt[:])
                    cs = cspool.tile([116, 512], bf16, tag="cs")
                    nc.vector.tensor_copy(cs[:], p_cs[:])
                    cs_tiles.append(cs)

                    # ---- |X| and gating input (sum over h) ----
                    csq = magpool.tile([52, 512], bf16, tag="csq")
                    nc.scalar.activation(csq[:], cs[0:52, :], Sq)
                    ssq = magpool.tile([52, 512], bf16, tag="ssq")
                    nc.gpsimd.tensor_tensor(ssq[:], cs[64:116, :], cs[64:116, :], MUL)
                    msum = magpool.tile([52, 512], bf16, tag="msum")
                    nc.gpsimd.tensor_tensor(msum[:], csq[:], ssq[:], ADD)
                    mag = magpool.tile([52, 512], bf16, tag="mmag")
                    nc.scalar.activation(
                        mag[:], msum[:], Sqrt, accum_out=gbuf[:, j : j + 1]
                    )

                # ---- gating MLP for the chunk ----
                # h1 = relu(W1^T (mean_h |X|) + b1): the half-fold and 1/H are
                # folded into w1f = [W1; W1] / H.
                p_h1 = ps_sm.tile([F, nb], f32, tag="sm")
                nc.tensor.matmul(p_h1[:], w1f[:], gbuf[:])
                h1 = gpool.tile([F, nb], f32, tag="h1")
                nc.scalar.activation(h1[:], p_h1[:], Relu, bias=b1[:])
                p_z = ps_sm.tile([E, nb], f32, tag="sm")
                nc.tensor.matmul(p_z[:], w2[:], h1[:])
                ez = gpool.tile([E, nb], f32, tag="ez")
                nc.scalar.activation(ez[:], p_z[:], Exp, bias=b2[:])
                p_s = ps_sm.tile([1, nb], f32, tag="sm")
                nc.tensor.matmul(p_s[:], ones8[:], ez[:])
                rs = gpool.tile([1, nb], f32, tag="rs")
                nc.vector.reciprocal(rs[:], p_s[:])
                p_r8 = ps_sm.tile([E, nb], f32, tag="sm")
                nc.tensor.matmul(p_r8[:], ones8r[:], rs[:])
                ezn = gpool.tile([E, nb], f32, tag="ezn")
                nc.vector.tensor_tensor(ezn[:], ez[:], p_r8[:], MUL)
                p_w = ps_sm.tile([F, nb], f32, tag="sm")
                nc.tensor.matmul(p_w[:], mask[:], ezn[:])
                w_sb = gpool.tile([F, nb], f32, tag="wsb")
                nc.vector.tensor_copy(w_sb[:], p_w[:])
                p_wrep = ps_sm.tile([116, nb], f32, tag="sm")
                nc.tensor.matmul(p_wrep[:], sel[:], w_sb[:])
                wrep = gpool.tile([116, nb], f32, tag="wrep")
                nc.vector.tensor_copy(wrep[:], p_wrep[:])

                # ---- inverse DFT with per-b scaled basis, write out ----
                for j in range(nb):
                    bb = c * nb + j
                    lo = lopool.tile([116, 114], bf16, tag="lo")
                    nc.gpsimd.tensor_scalar(
                        lo[:], abb[:], wrep[:, j : j + 1], None, MUL
                    )
                    p_o = ps_out.tile([114, 512], f32, tag="pout")
                    nc.tensor.matmul(p_o[:], lo[:], cs_tiles[j][:])
                    osb = opool.tile([114, 512], f32, tag="osb")
                    nc.scalar.activation(osb[:], p_o[:], Copy)
                    dsts = out_d[bb].rearrange(
                        "l (t half n) -> l t half n", t=4, half=2, n=128
                    )
                    nc.sync.dma_start(
                        out=dsts[:, :, 0, :],
                        in_=osb[0:50, :].rearrange("l (t n) -> l t n", t=4, n=128),
                    )
                    nc.sync.dma_start(
                        out=dsts[:, :, 1, :],
                        in_=osb[64:114, :].rearrange("l (t n) -> l t n", t=4, n=128),
                    )

    nc.compile()
    return nc


def _get_program(bs=BS, nb=NB):
    key = (bs, nb)
    if key not in _cache:
        _cache[key] = _build_program(bs, nb)
    return _cache[key]


def _host_consts(band_boundaries, W1, b1, W2, b2):
    import ml_dtypes

    bf = ml_dtypes.bfloat16
    R, I, A, B2 = _dft_consts()
    sig = 1.0 / (1.0 + np.exp(-band_boundaries.astype(np.float64)))
    bounds = np.concatenate([[0.0], np.sort(sig), [1.0]])
    idx = (bounds * F).astype(np.int32)
    idx[-1] = F
    k = np.arange(F)
    mask = (
        (k[None, :] >= idx[:-1, None]) & (k[None, :] < idx[1:, None])
    ).astype(np.float32)
    w1f = np.concatenate([W1, W1], axis=0).astype(np.float64) / H
    return {
        "ident": np.eye(128, dtype=np.float32).astype(bf),
        "ri2": _build_ri2(R, I).astype(np.float32).astype(bf),
        "abb": _build_abb(A, B2).astype(np.float32).astype(bf),
        "w1f": w1f.astype(np.float32),
        "b1c": b1.reshape(F, 1).astype(np.float32),
        "w2": W2.astype(np.float32),
        "b2c": b2.reshape(E, 1).astype(np.float32),
        "mask": mask,
        "ones8": np.ones((E, 1), np.float32),
        "ones8r": np.ones((1, E), np.float32),
        "sel": _build_sel(),
    }


def kernel(x, band_boundaries, W1, b1, W2, b2):
    from concourse.bass_utils import run_bass_kernel_spmd

    nc = _get_program()
    consts = _host_consts(
        np.asarray(band_boundaries), np.asarray(W1), np.asarray(b1),
        np.asarray(W2), np.asarray(b2),
    )
    x = np.ascontiguousarray(np.asarray(x, dtype=np.float32))
    in_maps = [
        {"x": x[i * BS : (i + 1) * BS], **consts} for i in range(N_CORES)
    ]
    res = run_bass_kernel_spmd(nc, in_maps, list(range(N_CORES)))
    return np.concatenate([res.results[i]["out"] for i in range(N_CORES)], axis=0)


# revision 20
# speedup vs baseline: 153.4873x; 153.4873x over previous
"""FAMoE layer Trainium2 kernel.

Math (per batch row b of x [B, H, L]):
  rfft over L is a matmul with fixed DFT bases R/I [L, F];
  gating = softmax(MLP(mean_h |X|)) @ band-mask -> per-frequency weight w [F];
  irfft(X * w) is a matmul with bases A/B2 [L, F] whose rows we pre-scale by w.
Output is [B, L, H] (transposed), which falls out naturally because the whole
pipeline runs in a frequency/L-on-partitions orientation:

  per b:  x tiles [128 rows, 2*50]  --PE transpose (matmul vs identity)-->
          xT [100, 512] (two 50-row h-blocks stacked)  --D matmul (RI2)-->
          C/S [116, 512] in PSUM (C at partitions 0-51, S at 64-115)
          --evac/square/add/sqrt+accum--> gating input [52, 1] per b
          ...chunk MLP...  --per-b scaled irfft basis (ABB*w) matmul-->
          out [114, 512] (l of h-block A at partitions 0-49, B at 64-113).

All partition-block bases are 0 or 64 (hardware requires 32-aligned bases).
Sharding: pure data parallel, batch dim split across 8 cores.
"""

import sys

sys.path.insert(0, "/opt/trn_rl_repo")

import numpy as np

N_CORES = 8
B, H, L = 2048, 1024, 50
F = 26
E = 8
BS = B // N_CORES          # 256 batch rows per core
NB = 16                    # MLP chunk size (batch rows per gating batch)
QUAD = 16                  # b rows per input DMA group (= NB)

_cache = {}


def _dft_consts():
    l = np.arange(L)[:, None].astype(np.float64)
    f = np.arange(F)[None, :].astype(np.float64)
    ang = 2.0 * np.pi * l * f / L
    R = np.cos(ang)                      # [L, F] rfft real basis
    I = -np.sin(ang)                     # [L, F] rfft imag basis
    c = np.full(F, 2.0)
    c[0] = 1.0
    c[F - 1] = 1.0
    A = c[None, :] * np.cos(ang) / L     # [L, F] irfft cos basis
    B2 = -c[None, :] * np.sin(ang) / L   # [L, F] irfft sin basis
    return R, I, A, B2


def _build_ri2(R, I):
    # lhsT for the forward DFT: [100, 116]
    # rows 0-49: l of h-block A; rows 50-99: l of h-block B
    # cols 0-25 C_A | 26-51 C_B | 52-63 zero | 64-89 S_A | 90-115 S_B
    RI2 = np.zeros((114, 128), np.float64)
    RI2[0:50, 0:26] = R
    RI2[64:114, 26:52] = R
    RI2[0:50, 64:90] = I
    RI2[64:114, 90:116] = I
    return RI2


def _build_abb(A, B2):
    # base lhsT for the inverse DFT: [116, 114]
    # rows (k) match the C/S PSUM partition layout; cols 0-49 out-l of block
    # A, 50-63 zero, 64-113 out-l of block B. Per-b this is row-scaled by w.
    ABB = np.zeros((116, 128), np.float64)
    ABB[0:26, 0:50] = A.T
    ABB[26:52, 64:114] = A.T
    ABB[64:90, 0:50] = B2.T
    ABB[90:116, 64:114] = B2.T
    return ABB


def _build_fold():
    # [116, 52] fold: msum[m] = sq[m] + sq[m + 64]  (C^2 + S^2 per (f, block))
    Fm = np.zeros((116, 52), np.float32)
    Fm[np.arange(52), np.arange(52)] = 1.0
    Fm[64 + np.arange(52), np.arange(52)] = 1.0
    return Fm


def _build_sel():
    # [26, 116] selector: wrep[q] = w[f(q)] for the four 26-blocks at 0/26/64/90
    S = np.zeros((26, 116), np.float32)
    for base in (0, 26, 64, 90):
        S[np.arange(26), base + np.arange(26)] = 1.0
    return S


def _build_program(bs, nb):
    from concourse import bacc, bass, mybir, tile

    f32 = mybir.dt.float32
    bf16 = mybir.dt.bfloat16

    nc = bacc.Bacc("TRN2", target_bir_lowering=False, debug=False)

    x_d = nc.dram_tensor("x", [bs, H, L], f32, kind="ExternalInput")
    out_d = nc.dram_tensor("out", [bs, L, H], f32, kind="ExternalOutput")
    ident_d = nc.dram_tensor("ident", [128, 128], bf16, kind="ExternalInput")
    ri2_d = nc.dram_tensor("ri2", [114, 128], bf16, kind="ExternalInput")
    abb_d = nc.dram_tensor("abb", [116, 128], bf16, kind="ExternalInput")
    w1f_d = nc.dram_tensor("w1f", [2 * F, F], f32, kind="ExternalInput")
    b1_d = nc.dram_tensor("b1c", [F, 1], f32, kind="ExternalInput")
    w2_d = nc.dram_tensor("w2", [F, E], f32, kind="ExternalInput")
    b2_d = nc.dram_tensor("b2c", [E, 1], f32, kind="ExternalInput")
    mask_d = nc.dram_tensor("mask", [E, F], f32, kind="ExternalInput")
    ones8_d = nc.dram_tensor("ones8", [E, 1], f32, kind="ExternalInput")
    ones8r_d = nc.dram_tensor("ones8r", [1, E], f32, kind="ExternalInput")
    sel_d = nc.dram_tensor("sel", [F, 116], f32, kind="ExternalInput")
    fold_d = nc.dram_tensor("fold", [116, 52], bf16, kind="ExternalInput")

    n_chunk = bs // nb
    assert bs % nb == 0 and nb == QUAD

    with tile.TileContext(nc) as tc:
        with (
            tc.tile_pool(name="consts", bufs=1) as cpool,
            tc.tile_pool(name="xin", bufs=3) as xpool,
            tc.tile_pool(name="xt", bufs=4) as xtpool,
            tc.tile_pool(name="cs", bufs=nb + 4) as cspool,
            tc.tile_pool(name="mag", bufs=3) as magpool,
            tc.tile_pool(name="outs", bufs=3) as opool,
            tc.tile_pool(name="gat", bufs=2) as gpool,
            tc.tile_pool(name="lo", bufs=3) as lopool,
            tc.tile_pool(name="ps_xt", bufs=3, space="PSUM") as ps_xt,
            tc.tile_pool(name="ps_cs", bufs=2, space="PSUM") as ps_cs,
            tc.tile_pool(name="ps_out", bufs=2, space="PSUM") as ps_out,
            tc.tile_pool(name="ps_sm", bufs=1, space="PSUM") as ps_sm,
        ):
            ident = cpool.tile([128, 128], bf16)
            ri2 = cpool.tile([114, 128], bf16)
            abb = cpool.tile([116, 128], bf16)
            w1f = cpool.tile([2 * F, F], f32)
            b1 = cpool.tile([F, 1], f32)
            w2 = cpool.tile([F, E], f32)
            b2 = cpool.tile([E, 1], f32)
            mask = cpool.tile([E, F], f32)
            ones8 = cpool.tile([E, 1], f32)
            ones8r = cpool.tile([1, E], f32)
            sel = cpool.tile([F, 116], f32)
            fold = cpool.tile([116, 52], bf16)
            for t, d in [
                (ident, ident_d), (ri2, ri2_d), (abb, abb_d), (w1f, w1f_d),
                (b1, b1_d), (w2, w2_d), (b2, b2_d), (mask, mask_d),
                (ones8, ones8_d), (ones8r, ones8r_d), (sel, sel_d),
                (fold, fold_d),
            ]:
                nc.sync.dma_start(t[:], d[:])

            Sq = mybir.ActivationFunctionType.Square
            Sqrt = mybir.ActivationFunctionType.Sqrt
            Copy = mybir.ActivationFunctionType.Copy
            Relu = mybir.ActivationFunctionType.Relu
            Exp = mybir.ActivationFunctionType.Exp
            MUL = mybir.AluOpType.mult
            ADD = mybir.AluOpType.add

            for c in range(n_chunk):
                gbuf = gpool.tile([52, nb], f32, tag="gbuf")
                cs_tiles = []
                for j in range(nb):
                    bb = c * nb + j
                    # ---- input DMA: whole chunk, one DMA per t-block ----
                    if j == 0:
                        x_nat = xpool.tile([128, QUAD * 512], bf16, tag="xnat")
                        # zero the l=50..63 pad columns (strided view, cheap)
                        pads = x_nat[:, :].rearrange(
                            "p (tb pair l) -> p tb pair l", tb=4 * QUAD, pair=2, l=64
                        )[:, :, :, 50:64]
                        nc.vector.memset(pads, 0.0)
                        for t in range(4):
                            srcv = x_d[bb : bb + QUAD].rearrange(
                                "b (pair t p) l -> p t b pair l", pair=2, t=4, p=128
                            )[:, t]
                            dstv = x_nat[
                                :, t * 2 * QUAD * 64 : (t + 1) * 2 * QUAD * 64
                            ].rearrange(
                                "p (b pair l) -> p b pair l", b=QUAD, pair=2, l=64
                            )[:, :, :, 0:50]
                            nc.gpsimd.dma_start(out=dstv, in_=srcv)

                    # ---- transpose via PE (x tile stationary, identity moving) ----
                    # stationary cols: 0-49 pair-A l, 50-63 zero pad, 64-113
                    # pair-B l, 114-127 zero pad -> out rows land at 0/64.
                    # ---- transpose via PE (x tile stationary, identity moving) ----
                    # stationary cols: 0-49 pair-A l, 50-63 zero pad, 64-113
                    # pair-B l, 114-127 zero pad -> out rows land at 0/64.
                    p_xt = ps_xt.tile([128, 512], f32, tag="pxt")
                    for t in range(4):
                        nc.tensor.matmul(
                            p_xt[:, 128 * t : 128 * t + 128],
                            x_nat[:, t * 2 * QUAD * 64 + j * 128 : t * 2 * QUAD * 64 + j * 128 + 128],
                            ident[:],
                        )
                    xt = xtpool.tile([114, 512], bf16, tag="xt")
                    nc.scalar.activation(xt[:], p_xt[0:114, :], Copy)

                    # ---- forward DFT ----
                    p_cs = ps_cs.tile([128, 512], f32, tag="pcs")
                    nc.tensor.matmul(p_cs[:], ri2[:], xt[:])
                    cs = cspool.tile([116, 512], bf16, tag="cs")
                    nc.vector.tensor_copy(cs[:], p_cs[0:116, :])
                    cs_tiles.append(cs)

                    # ---- |X| and gating input (sum over h) ----
                    sq116 = magpool.tile([116, 512], bf16, tag="sq116")
                    nc.vector.tensor_tensor(sq116[:], cs[:], cs[:], MUL)
                    p_fold = ps_sm.tile([52, 512], f32, tag="sm")
                    nc.tensor.matmul(p_fold[:], fold[:], sq116[:])
                    mag = magpool.tile([52, 512], bf16, tag="mmag")
                    nc.scalar.activation(
                        mag[:], p_fold[:], Sqrt, accum_out=gbuf[:, j : j + 1]
                    )

                # ---- gating MLP for the chunk ----
                # h1 = relu(W1^T (mean_h |X|) + b1): the half-fold and 1/H are
                # folded into w1f = [W1; W1] / H.
                p_h1 = ps_sm.tile([F, nb], f32, tag="sm")
                nc.tensor.matmul(p_h1[:], w1f[:], gbuf[:])
                h1 = gpool.tile([F, nb], f32, tag="h1")
                nc.scalar.activation(h1[:], p_h1[:], Relu, bias=b1[:])
                p_z = ps_sm.tile([E, nb], f32, tag="sm")
                nc.tensor.matmul(p_z[:], w2[:], h1[:])
                ez = gpool.tile([E, nb], f32, tag="ez")
                nc.scalar.activation(ez[:], p_z[:], Exp, bias=b2[:])
                p_s = ps_sm.tile([1, nb], f32, tag="sm")
                nc.tensor.matmul(p_s[:], ones8[:], ez[:])
                rs = gpool.tile([1, nb], f32, tag="rs")
                nc.vector.reciprocal(rs[:], p_s[:])
                p_r8 = ps_sm.tile([E, nb], f32, tag="sm")
                nc.tensor.matmul(p_r8[:], ones8r[:], rs[:])
                ezn = gpool.tile([E, nb], f32, tag="ezn")
                nc.vector.tensor_tensor(ezn[:], ez[:], p_r8[:], MUL)
                p_w = ps_sm.tile([F, nb], f32, tag="sm")
                nc.tensor.matmul(p_w[:], mask[:], ezn[:])
                w_sb = gpool.tile([F, nb], f32, tag="wsb")
                nc.vector.tensor_copy(w_sb[:], p_w[:])
                p_wrep = ps_sm.tile([116, nb], f32, tag="sm")
                nc.tensor.matmul(p_wrep[:], sel[:], w_sb[:])
                wrep = gpool.tile([116, nb], f32, tag="wrep")
                nc.vector.tensor_copy(wrep[:], p_wrep[:])

                # ---- inverse DFT with per-b scaled basis, write out ----
                for j in range(nb):
                    bb = c * nb + j
                    lo = lopool.tile([116, 128], bf16, tag="lo")
                    nc.vector.tensor_scalar(
                        lo[:], abb[:], wrep[:, j : j + 1], None, MUL
                    )
                    p_o = ps_out.tile([128, 512], f32, tag="pout")
                    nc.tensor.matmul(p_o[:], lo[:], cs_tiles[j][:])
                    if j % 2 == 0:
                        osb = opool.tile([114, 1024], f32, tag="osb")
                    half = osb[:, 512 * (j % 2) : 512 * (j % 2) + 512]
                    if j % 8 < 5:
                        nc.scalar.activation(half, p_o[0:114, :], Copy)
                    else:
                        nc.vector.tensor_copy(half, p_o[0:114, :])
                    if j % 2 == 1:
                        dstA = out_d[bb - 1 : bb + 1, :, 0:512].rearrange(
                            "b l n -> l b n"
                        )
                        dstB = out_d[bb - 1 : bb + 1, :, 512:1024].rearrange(
                            "b l n -> l b n"
                        )
                        nc.sync.dma_start(
                            out=dstA,
                            in_=osb[0:50, :].rearrange("l (b n) -> l b n", b=2, n=512),
                        )
                        nc.sync.dma_start(
                            out=dstB,
                            in_=osb[64:114, :].rearrange("l (b n) -> l b n", b=2, n=512),
                        )

    nc.compile()
    return nc


def _get_program(bs=BS, nb=NB):
    key = (bs, nb)
    if key not in _cache:
        _cache[key] = _build_program(bs, nb)
    return _cache[key]


def _host_consts(band_boundaries, W1, b1, W2, b2):
    import ml_dtypes

    bf = ml_dtypes.bfloat16
    R, I, A, B2 = _dft_consts()
    sig = 1.0 / (1.0 + np.exp(-band_boundaries.astype(np.float64)))
    bounds = np.concatenate([[0.0], np.sort(sig), [1.0]])
    idx = (bounds * F).astype(np.int32)
    idx[-1] = F
    k = np.arange(F)
    mask = (
        (k[None, :] >= idx[:-1, None]) & (k[None, :] < idx[1:, None])
    ).astype(np.float32)
    w1f = np.concatenate([W1, W1], axis=0).astype(np.float64) / H
    return {
        "ident": np.eye(128, dtype=np.float32).astype(bf),
        "ri2": _build_ri2(R, I).astype(np.float32).astype(bf),
        "abb": _build_abb(A, B2).astype(np.float32).astype(bf),
        "w1f": w1f.astype(np.float32),
        "b1c": b1.reshape(F, 1).astype(np.float32),
        "w2": W2.astype(np.float32),
        "b2c": b2.reshape(E, 1).astype(np.float32),
        "mask": mask,
        "ones8": np.ones((E, 1), np.float32),
        "ones8r": np.ones((1, E), np.float32),
        "sel": _build_sel(),
        "fold": _build_fold().astype(bf),
    }


def kernel(x, band_boundaries, W1, b1, W2, b2):
    from concourse.bass_utils import run_bass_kernel_spmd

    nc = _get_program()
    consts = _host_consts(
        np.asarray(band_boundaries), np.asarray(W1), np.asarray(b1),
        np.asarray(W2), np.asarray(b2),
    )
    x = np.ascontiguousarray(np.asarray(x, dtype=np.float32))
    in_maps = [
        {"x": x[i * BS : (i + 1) * BS], **consts} for i in range(N_CORES)
    ]
    res = run_bass_kernel_spmd(nc, in_maps, list(range(N_CORES)))
    return np.concatenate([res.results[i]["out"] for i in range(N_CORES)], axis=0)
